# revision 1
# baseline (speedup 1.0000x reference)
"""Trainium2 Bass kernel for nn_ECA_69544110457542.

Math (per row r=(b,t)):
  dyn   = x[:, :31] @ Wd + bd
  value = x[:, 31] * Wv[0] + bv
  xhn   = [dyn | human@Wh+bh | nature@Wn+bn]                      (768 ch)
  pre_j = sum_k cw[t,k] * xhn[perm[ainv[j]+k-3]] + conv_b[t]      (j<256)
  sel   = softmax(relu(pre))
  out   = 0.5*(dyn*sel) @ Wvd1  +  0.5*dyn @ Wvd1 + value @ Wvd2 + bvd
          `------ device ------'  `------- folded into Wfold (host) -----'

Key folds / layout choices:
  - channel shuffle + depthwise-conv gather -> permuted weight matrix Wg
    [193, 1280] whose column (k*256+j) reproduces xhn[:, perm[ainv[j]+k-3]];
  - all purely-linear output terms -> Wfold (one K=33 fp32 matmul);
  - activations stored PRE-TRANSPOSED in DRAM (bf16 for the softmax path,
    fp32 x for the linear path) so lhsT tiles load straight off DMA;
  - the whole softmax path (a <= few % correction of the output) runs in
    bf16: G matmuls, conv combine, exp, gate, z @ Wvd1;
  - per-t conv weights cw[t,k] are applied FREE inside the ACT-engine PSUM
    drains (activation scale= is a per-partition AP), so the DVE combine is
    just 4 bf16 adds;
  - exp's accum_out gives the softmax denominator for free;
  - the 0.5 gate factor is folded into Wvd1.

Sharding: pure data parallel, 32 batches per core on 8 cores.
"""

import sys

sys.path.insert(0, "/opt/trn_rl_repo")

from contextlib import ExitStack

import ml_dtypes
import numpy as np

import concourse.bass as bass
import concourse.tile as tile
from concourse import mybir
from concourse.tile import add_dep_helper
from concourse.bass_utils import run_bass_kernel_spmd

# problem constants
B, T, E = 256, 64, 256
XS, DS = 32, 31
HT, NT_ = 80, 80
C = 3 * E
KW = 5
NCORES = 8
BPC = B // NCORES          # 32 batches per core
R = BPC * T                # 2048 rows per core
P = 128
NTILES = R // P            # 16
AK = XS + 1 + HT + NT_     # 193 act rows: x(32) | ones | h(80) | n(80)
K2 = AK - 128              # 65
NG = KW * E                # 1280 gathered columns
NA = 640                   # G psum half A: slices k0,k1,k2[:128]
NB = 640                   # half B: k2[128:],k3,k4

# packed-constants layout, fp32 slot offsets in [128, WPACK]
O_WG1 = 0                   # bf16 [128, 1280] -> 640 slots
O_WG2 = O_WG1 + NG // 2     # 640:  bf16 [65, 1280] -> 640 slots
O_WDYN = O_WG2 + NG // 2    # 1280: bf16 [33, 256] -> 128 slots
O_WFOLD = O_WDYN + E // 2   # 1408: fp32 [33, 256]
O_WV1 = O_WFOLD + E         # 1664: bf16 0.5*Wvd1 [256, 256] -> 256 slots
O_IDB = O_WV1 + E           # 1920: bf16 identity -> 64 slots
O_CW = O_IDB + P // 2       # 1984: fp32 [128, 5]
O_CB = O_CW + KW            # 1989: fp32 [128, 1]
WPACK = O_CB + 1            # 1990

F32 = mybir.dt.float32
BF16 = mybir.dt.bfloat16
MULT = mybir.AluOpType.mult
ADD = mybir.AluOpType.add
IDENT = mybir.ActivationFunctionType.Identity

_NC_CACHE = None
LAST_RESULTS = None
TRACE = False


def _patched_drain_and_barrier(self, tick_clock, wait_clock):
    # The stock kernel-tail drain puts every processor's final-tick wait on a
    # single Drain instruction; this walrus build rejects multi-wait
    # instructions, so spread the waits over a chain of drains instead.
    import bass_rust as _br
    from concourse.vector_clock import ScopedClock

    nc = self.nc
    drain_inst = nc.sync.drain()
    wait_clock.add_sem_waits(
        drain_inst.ins, ScopedClock({None: tick_clock.global_clock})
    )
    si = drain_inst.ins.sync_info
    if si is not None and len(si.on_wait) > 1:
        waits = list(si.on_wait)
        drain_inst.ins.sync_info = _br.SyncInfo(
            on_wait=[waits[0]], on_update=list(si.on_update)
        )
        for w in waits[1:]:
            d2 = nc.sync.drain()
            d2.ins.sync_info = _br.SyncInfo(on_wait=[w], on_update=[])
    nc.all_engine_barrier()
    assert self.sems is not None
    popped = nc._tile_sem_poison_stack.pop()
    assert popped is self._sem_poison
    nc.clear_and_free_semaphores(list(self.sems.allocated().values()))
    nc.all_engine_barrier()


tile.TileContext._drain_and_barrier = _patched_drain_and_barrier


def _build_nc():
    nc = bass.Bass()
    actb_d = nc.dram_tensor("actb", [P, 2 * R], BF16, kind="ExternalInput")
    actx_d = nc.dram_tensor("actx", [33, R], F32, kind="ExternalInput")
    wpack_d = nc.dram_tensor("wpack", [P, WPACK], F32, kind="ExternalInput")
    out_d = nc.dram_tensor("out", [R, E], F32, kind="ExternalOutput")
    actb3 = actb_d[:, :].rearrange("p (two r) -> p two r", two=2)

    with tile.TileContext(nc) as tc, ExitStack() as ctx:
        consts = ctx.enter_context(tc.tile_pool(name="consts", bufs=1))
        pactB = ctx.enter_context(tc.tile_pool(name="pactB", bufs=16))
        pactX = ctx.enter_context(tc.tile_pool(name="pactX", bufs=16))
        pgs = ctx.enter_context(tc.tile_pool(name="pgs", bufs=3))
        pacc = ctx.enter_context(tc.tile_pool(name="pacc", bufs=8))
        pex = ctx.enter_context(tc.tile_pool(name="pex", bufs=4))
        psml = ctx.enter_context(tc.tile_pool(name="psml", bufs=8))
        pz = ctx.enter_context(tc.tile_pool(name="pz", bufs=4))
        pzT = ctx.enter_context(tc.tile_pool(name="pzT", bufs=4))
        pot = ctx.enter_context(tc.tile_pool(name="pot", bufs=4))
        ptch = ctx.enter_context(tc.tile_pool(name="ptch", bufs=4))
        pG = ctx.enter_context(tc.tile_pool(name="pG", bufs=2, space="PSUM"))
        pdf = ctx.enter_context(tc.tile_pool(name="pdf", bufs=2, space="PSUM"))
        ptz = ctx.enter_context(tc.tile_pool(name="ptz", bufs=2, space="PSUM"))

        wp = consts.tile([P, WPACK], F32)
        nc.sync.dma_start(wp[:], wpack_d[:, :])
        wpb = wp[:].bitcast(BF16)
        wg1 = wpb[:, 2 * O_WG1 : 2 * O_WG1 + NG]
        wg2 = wpb[0:K2, 2 * O_WG2 : 2 * O_WG2 + NG]
        wdyn = wpb[0:33, 2 * O_WDYN : 2 * O_WDYN + E]
        wfold = wp[0:33, O_WFOLD : O_WFOLD + E]
        wv1a = wpb[:, 2 * O_WV1 : 2 * O_WV1 + E]
        wv1b = wpb[:, 2 * O_WV1 + E : 2 * O_WV1 + 2 * E]
        identb = wpb[:, 2 * O_IDB : 2 * O_IDB + P]
        cw = wp[:, O_CW : O_CW + KW]
        cb = wp[:, O_CB : O_CB + 1]

        # PE/DVE observe the weights DMA once (compute instructions carry
        # only ONE sem-wait on this walrus build)
        scr = ptz.tile([P, 1], F32, tag="ptz")
        nc.tensor.matmul(scr[:], identb, identb[:, 0:1], start=True, stop=True)
        wtouch = psml.tile([P, 1], F32, tag="sml")
        nc.vector.tensor_copy(wtouch[:], cb)
        stouch = psml.tile([P, 1], F32, tag="sml")
        nc.scalar.copy(stouch[:], cb)

        z_prev = None
        z_prev2 = None
        gs_prev = None
        mmg_last_prev = None
        mmz_prev = None
        mmz_prev2 = None
        obufs = {}
        pend = None

        def flush_z_impl(pend):
            # z-path of tile j, emitted one iteration later so PE never
            # waits on the current tile's softmax chain
            j, zj, pdfj = pend
            ptz_t = ptz.tile([P, 2, P], BF16, tag="ptz")
            nc.tensor.transpose(ptz_t[:, 0, :], zj[:, 0:128], identb)
            nc.tensor.transpose(ptz_t[:, 1, :], zj[:, 128:256], identb)
            zT = pzT.tile([P, 2, P], BF16)
            nc.vector.tensor_copy(zT[:], ptz_t[:])
            nc.tensor.matmul(
                pdfj[:, 256:512], zT[:, 0, :], wv1a,
                start=False, stop=False, skip_group_check=True,
            )
            mmz2 = nc.tensor.matmul(
                pdfj[:, 256:512], zT[:, 1, :], wv1b,
                start=False, stop=True, skip_group_check=True,
            )
            # ACT observes the gate's DVE tick before the out-copy
            zt_ = ptch.tile([1, 2], BF16, tag="tch2")
            nc.scalar.copy(zt_[:], zj[0:1, 0:2])
            if j % 4 == 0:
                obufs[j // 4] = pot.tile([P, 4, E], F32, tag="obuf", name=f"obuf{j // 4}")
            ob = obufs[j // 4]
            nc.scalar.copy(ob[:, j % 4, :], pdfj[:, 256:512])
            if j % 4 == 3:
                g0 = (j - 3) * P
                odst = out_d[g0 : g0 + 4 * P, :].rearrange(
                    "(t p) e -> p t e", p=P
                )
                nc.gpsimd.dma_start(odst, ob[:])
            return mmz2

        for i in range(NTILES):
            pend_prev = pend
            rows = slice(i * P, (i + 1) * P)
            actb = pactB.tile([P, 2, P], BF16)
            nc.sync.dma_start(actb[:], actb3[:, :, rows])
            actx = pactX.tile([33, P], F32)
            nc.sync.dma_start(actx[:], actx_d[:, rows])

            # "PE observes processor X" gadgets: every real matmul self-loads
            # its weights, so stray LDWEIGHTS are harmless
            absorbers = [
                nc.tensor.ldweights(actb[0:1, 0, 0:2]),
                nc.tensor.ldweights(actx[:].bitcast(BF16)[0:1, 0:2]),
            ]
            if z_prev2 is not None:
                absorbers.append(nc.tensor.ldweights(z_prev2[0:1, 0:2]))
            if gs_prev is not None:
                absorbers.append(nc.tensor.ldweights(gs_prev[0:1, NG - 2 : NG]))
            if mmg_last_prev is not None:
                ldw_self = nc.tensor.ldweights(wpb[0:1, 0:2])
                add_dep_helper(ldw_self.ins, mmg_last_prev.ins, sync=True,
                               reason="absorb PE W-W completion wait")
                if mmz_prev2 is not None:
                    add_dep_helper(ldw_self.ins, mmz_prev2.ins, sync=True,
                                   reason="absorb PE W-W completion wait")
                absorbers.append(ldw_self)

            # dyn (bf16) and folded-linear out part (fp32)
            pdf_t = pdf.tile([P, 512], F32)
            mm_df = nc.tensor.matmul(
                pdf_t[:, 0:E], actb[0:33, 0, :], wdyn, start=True, stop=True
            )
            mm_fo = nc.tensor.matmul(
                pdf_t[:, E:512], actx[:], wfold, start=True, stop=True
            )
            for a in absorbers:
                add_dep_helper(mm_df.ins, a.ins, sync=False,
                               reason="absorbers run before first matmul")
                add_dep_helper(mm_fo.ins, a.ins, sync=False,
                               reason="absorbers run before first matmul")

            # gathered conv operand columns, two psum halves
            pGA = pG.tile([P, NA], F32, tag="G")
            pGB = pG.tile([P, NB], F32, tag="G")
            for gt, c0 in ((pGA, 0), (pGB, NA)):
                for s0, s1 in ((0, 512), (512, 640)):
                    mg = nc.tensor.matmul(
                        gt[:, s0:s1], actb[:, 0, :], wg1[:, c0 + s0 : c0 + s1],
                        start=True, stop=False,
                    )
                    if s0 == 0:
                        for a in absorbers:
                            add_dep_helper(mg.ins, a.ins, sync=False,
                                           reason="absorbers first")
                    mmg_last_prev = nc.tensor.matmul(
                        gt[:, s0:s1], actb[0:K2, 1, :], wg2[:, c0 + s0 : c0 + s1],
                        start=False, stop=True,
                    )


            # ACT observes half A's matmuls, drains A, then B — so drain-A
            # overlaps B's matmuls and the DVE chain starts a drain earlier
            pgtA = ptch.tile([1, 2], BF16, tag="tch2")
            nc.scalar.copy(pgtA[:], pGA[:].bitcast(BF16)[0:1, 0:2])
            gs = pgs.tile([P, NG], BF16)
            nc.scalar.copy(gs[:, 0:NA], pGA[:])
            pgtB = ptch.tile([1, 2], BF16, tag="tch2")
            nc.scalar.copy(pgtB[:], pGB[:].bitcast(BF16)[0:1, 0:2])
            nc.scalar.copy(gs[:, NA:NG], pGB[:])

            # conv combine: scale each k-slice by cw[t,k] (4x-mode
            # tensor_scalar), conv_b folded into the k0 scale op, then add.
            # DVE touches each drain once (single-wait rule) and works on
            # half A while ACT is still draining half B.
            gtA = ptch.tile([1, 2], BF16, tag="tch")
            nc.vector.tensor_copy(gtA[:], gs[0:1, NA - 2 : NA])
            g0 = pacc.tile([P, E], BF16, tag="acc")
            nc.vector.tensor_scalar(g0[:], gs[:, 0:256], cw[:, 0:1], cb,
                                    op0=MULT, op1=ADD)
            g1 = pacc.tile([P, E], BF16, tag="acc")
            nc.vector.tensor_scalar_mul(g1[:], gs[:, 256:512], cw[:, 1:2])
            a01 = pacc.tile([P, E], BF16, tag="acc")
            nc.vector.tensor_add(a01[:], g0[:], g1[:])
            gtB = ptch.tile([1, 2], BF16, tag="tch")
            nc.vector.tensor_copy(gtB[:], gs[0:1, NG - 2 : NG])
            g2 = pacc.tile([P, E], BF16, tag="acc")
            nc.vector.tensor_scalar_mul(g2[:], gs[:, 512:768], cw[:, 2:3])
            g3 = pacc.tile([P, E], BF16, tag="acc")
            nc.vector.tensor_scalar_mul(g3[:], gs[:, 768:1024], cw[:, 3:4])
            g4 = pacc.tile([P, E], BF16, tag="acc")
            nc.vector.tensor_scalar_mul(g4[:], gs[:, 1024:1280], cw[:, 4:5])
            a34 = pacc.tile([P, E], BF16, tag="acc")
            nc.vector.tensor_add(a34[:], g3[:], g4[:])
            a0134 = pacc.tile([P, E], BF16, tag="acc")
            nc.vector.tensor_add(a0134[:], a01[:], a34[:])
            pre = pacc.tile([P, E], BF16, tag="acc")
            nc.vector.tensor_add(pre[:], a0134[:], g2[:])
            relu = pacc.tile([P, E], BF16, tag="acc")
            nc.vector.tensor_scalar_max(relu[:], pre[:], 0.0)

            # exp + free row-sum via accum_out
            exm = pex.tile([P, E], BF16, tag="exm")
            ssum = psml.tile([P, 1], F32, tag="sml")
            nc.scalar.activation(
                exm[:], relu[:], func=mybir.ActivationFunctionType.Exp,
                accum_out=ssum[:],
            )
            sinv = psml.tile([P, 1], F32, tag="sml")
            nc.vector.reciprocal(sinv[:], ssum[:])

            # DVE observes the dyn/fold matmuls once before the gate
            pdtouch = ptch.tile([1, 2], BF16, tag="tch")
            nc.vector.tensor_copy(pdtouch[:], pdf_t[:].bitcast(BF16)[0:1, 0:2])

            # z = (exm / S) * dyn  (the 0.5 is folded into Wvd1)
            z = pz.tile([P, E], BF16, tag="z")
            nc.vector.scalar_tensor_tensor(
                z[:], exm[:], sinv[:], pdf_t[:, 0:E], op0=MULT, op1=MULT
            )
            z_prev2 = z_prev
            z_prev = z
            gs_prev = gs
            pend = (i, z, pdf_t)

            flush_z_impl(pend)

    return nc


def _host_prep(x, human, nature, perm, Wv, bv, Wd, bd, Wh, bh, Wn, bn,
               conv_w, conv_b, Wvd, bvd):
    f = np.float32
    bf = ml_dtypes.bfloat16
    x = np.asarray(x, f)
    human = np.asarray(human, f)
    nature = np.asarray(nature, f)
    Wv = np.asarray(Wv, f); bv = np.asarray(bv, f)
    Wd = np.asarray(Wd, f); bd = np.asarray(bd, f)
    Wh = np.asarray(Wh, f); bh = np.asarray(bh, f)
    Wn = np.asarray(Wn, f); bn = np.asarray(bn, f)
    conv_w = np.asarray(conv_w, f)
    conv_b = np.asarray(conv_b, f)
    Wvd = np.asarray(Wvd, f); bvd = np.asarray(bvd, f)
    perm = np.asarray(perm).astype(np.int64)

    Wvd1 = Wvd[:E, :]
    Wvd2 = Wvd[E:, :]

    acts = np.concatenate(
        [
            x.reshape(B * T, XS),
            np.ones((B * T, 1), f),
            human.reshape(B * T, HT),
            nature.reshape(B * T, NT_),
        ],
        axis=1,
    )
    actsT = np.ascontiguousarray(acts.T)  # [193, B*T]
    actb = np.zeros((P, 2, B * T), bf)
    actb[:, 0, :] = actsT[0:128]
    actb[0:K2, 1, :] = actsT[128:AK]
    actx = np.ascontiguousarray(actsT[0:33])  # fp32 [33, B*T]

    wpack = np.zeros((P, WPACK), f)
    wpv = wpack.view(bf)  # bf16 alias [128, 2*WPACK]

    # folded linear path (fp32)
    wfold = np.zeros((33, E), f)
    wfold[0:DS] = 0.5 * (Wd @ Wvd1)
    wfold[31] = Wv[0] @ Wvd2
    wfold[32] = 0.5 * (bd @ Wvd1) + bv @ Wvd2 + bvd
    wpack[0:33, O_WFOLD : O_WFOLD + E] = wfold

    # dyn (bf16)
    wdyn = np.zeros((33, E), f)
    wdyn[0:DS] = Wd
    wdyn[32] = bd
    wpv[0:33, 2 * O_WDYN : 2 * O_WDYN + E] = wdyn.astype(bf)

    # gathered conv weights (bf16)
    ainv = np.argsort(perm)
    Wg = np.zeros((AK, NG), f)
    for k in range(KW):
        pos = ainv[:E] + k - 3
        for j in range(E):
            pj = pos[j]
            if 0 <= pj < C:
                c = perm[pj]
                col = k * E + j
                if c < E:
                    Wg[0:DS, col] = Wd[:, c]
                    Wg[32, col] = bd[c]
                elif c < 2 * E:
                    Wg[33:113, col] = Wh[:, c - E]
                    Wg[32, col] = bh[c - E]
                else:
                    Wg[113:193, col] = Wn[:, c - 2 * E]
                    Wg[32, col] = bn[c - 2 * E]
    wpv[:, 2 * O_WG1 : 2 * O_WG1 + NG] = Wg[0:128].astype(bf)
    wpv[0:K2, 2 * O_WG2 : 2 * O_WG2 + NG] = Wg[128:AK].astype(bf)

    # 0.5 * Wvd1 (bf16), split into two K-chunks
    wv1 = (0.5 * Wvd1).astype(bf)
    wpv[:, 2 * O_WV1 : 2 * O_WV1 + E] = wv1[0:128]
    wpv[:, 2 * O_WV1 + E : 2 * O_WV1 + 2 * E] = wv1[128:256]

    wpv[:, 2 * O_IDB : 2 * O_IDB + P] = np.eye(P, dtype=bf)
    wpack[:, O_CW : O_CW + KW] = np.tile(conv_w[:, 0, :], (2, 1))
    wpack[:, O_CB] = np.tile(conv_b, 2)
    return actb, actx, wpack


def kernel(**inputs):
    global _NC_CACHE, LAST_RESULTS
    actb, actx, wpack = _host_prep(**inputs)

    if _NC_CACHE is None:
        _NC_CACHE = _build_nc()
    nc = _NC_CACHE

    in_maps = []
    for ci in range(NCORES):
        sb = np.ascontiguousarray(actb[:, :, ci * R : (ci + 1) * R]).reshape(
            P, 2 * R
        )
        sx = np.ascontiguousarray(actx[:, ci * R : (ci + 1) * R])
        in_maps.append({"actb": sb, "actx": sx, "wpack": wpack})

    res = run_bass_kernel_spmd(nc, in_maps, core_ids=list(range(NCORES)), trace=TRACE)
    LAST_RESULTS = res

    out = np.empty((B, T, E), np.float32)
    for ci in range(NCORES):
        out[ci * BPC : (ci + 1) * BPC] = res.results[ci]["out"].reshape(BPC, T, E)
    return out

